# revision 5
# baseline (speedup 1.0000x reference)
"""Trainium2 Bass kernel for nn_DCSRM — v2 (dense-PE rewrite).

Identity: conv(x*g, w) = conv(x, w*g) — g folds into the 9 tap weights.

Per core: 2 samples x 2 channel-halves = 4 tiles [128ch, 99*96] f32r.
Conv split by output row:
  - PE rows [0, RPE): fp32r diag matmuls, 512-col flat chunks, 9 taps
    accumulated per psum bank (chunk-major), 4-bank rotation; Scalar
    drains psum -> contiguous staging; DVE fixes column-wrap
    contamination; DMA out per ~24-row group.
  - DVE rows [RPE, 96): Scalar center-tap scaled-copy initializes
    staging, DVE adds 8 taps via scalar_tensor_tensor (exact col-split
    APs, no fixups needed).
Stats (sum/sumsq per channel): per-tile split — DVE bn_stats on the
first fraction, Scalar ACT accum (Copy/Square) on the rest, emitted
per-quarter as input DMAs land. Scalar fillers (stats pieces, dg
builds) interleave between drains so PE never stalls.
"""
import os
import sys
import types
import contextlib
from contextlib import ExitStack

sys.path.insert(0, '/opt/trn_rl_repo')

import numpy as np

N, C, H, W = 16, 256, 96, 96
EPS = 1e-5
NPIX = H * W                      # 9216
CORES = 8
S_PER_CORE = N // CORES           # 2
HALVES = C // 128                 # 2

GUARD_TOP = 2
GUARD_BOT = 1
ROWS_BUF = GUARD_TOP + H + GUARD_BOT          # 99
XT_LEN = ROWS_BUF * W                          # 9504
DATA_OFF = GUARD_TOP * W                       # 192

RPE = 72                  # PE-owned rows per tile; DVE owns 96-RPE
RDVE = H - RPE
CHUNK = 384               # psum chunk cols (4 rows, fits 2KB bank)
DMA_ROWS = 24             # PE-region out-DMA/fixup group size (rows)
GCOLS = DMA_ROWS * W      # 2304 cols per stage group (6 chunks)

LAST_EXEC_NS = None
LAST_RESULTS = None
_PROGRAM_CACHE = {}


def _install_trace_hook_shim():
    try:
        import antenv.axon_hooks  # noqa: F401
        return
    except ImportError:
        pass
    try:
        import antenv
        import ctypes
    except ImportError:
        return
    so_path = '/opt/axon/libaxon_pjrt.so'

    def _build():
        if not os.path.exists(so_path):
            return None
        lib = ctypes.CDLL(so_path)
        if not hasattr(lib, 'axon_start_nrt_profile'):
            return None
        lib.axon_start_nrt_profile.argtypes = [
            ctypes.POINTER(ctypes.c_int64), ctypes.c_size_t]
        lib.axon_start_nrt_profile.restype = ctypes.c_int64
        lib.axon_stop_nrt_profile.argtypes = [ctypes.c_char_p]
        lib.axon_stop_nrt_profile.restype = ctypes.c_int64

        @contextlib.contextmanager
        def _hook(output_dir, device_ids):
            import jax
            jax.devices()
            if device_ids:
                ids = (ctypes.c_int64 * len(device_ids))(*device_ids)
                rc = lib.axon_start_nrt_profile(ids, len(device_ids))
            else:
                rc = lib.axon_start_nrt_profile(None, 0)
            if rc != 0:
                raise RuntimeError(f'axon_start_nrt_profile rc={rc}')
            try:
                yield
            finally:
                n = lib.axon_stop_nrt_profile(str(output_dir).encode())
                print(f'ntff profile: {n} file(s) -> {output_dir}',
                      file=sys.stderr)
        return _hook

    mod = types.ModuleType('antenv.axon_hooks')
    holder = {'hook': _build()}
    mod.get_axon_ntff_profile_hook = lambda: holder['hook']
    mod.set_axon_ntff_profile_hook = lambda h: holder.update(hook=h)
    sys.modules['antenv.axon_hooks'] = mod
    antenv.axon_hooks = mod


def _tap(k):
    return k // 3 - 1, k % 3 - 1          # dy, dx


def _build_program():
    from concourse import bacc, mybir, tile

    F32 = mybir.dt.float32
    F32R = mybir.dt.float32r
    OP = mybir.AluOpType
    AF = mybir.ActivationFunctionType

    nc = bacc.Bacc('TRN2', target_bir_lowering=False, debug=False,
                   num_devices=CORES)

    x_d = nc.dram_tensor('x', [S_PER_CORE, C, H, W], F32R,
                         kind='ExternalInput').ap()
    ca_d = nc.dram_tensor('constA', [128, 183], F32,
                          kind='ExternalInput').ap()
    cb_d = nc.dram_tensor('constB', [16, 2305], F32,
                          kind='ExternalInput').ap()
    out_d = nc.dram_tensor('out', [S_PER_CORE, C, H, W], F32,
                           kind='ExternalOutput').ap()

    with tile.TileContext(nc) as tc:
        with ExitStack() as ctx:
            cpool = ctx.enter_context(tc.tile_pool(name='const', bufs=1))
            xpool = ctx.enter_context(tc.tile_pool(name='x', bufs=4))
            stpool = ctx.enter_context(tc.tile_pool(name='stage', bufs=2))
            svpool = ctx.enter_context(tc.tile_pool(name='stagev', bufs=1))
            bnpool = ctx.enter_context(tc.tile_pool(name='bn', bufs=1))
            dumppool = ctx.enter_context(tc.tile_pool(name='dump', bufs=1))
            smpool = ctx.enter_context(tc.tile_pool(name='small', bufs=1))
            dgpool = ctx.enter_context(tc.tile_pool(name='diag', bufs=2))
            pspool = ctx.enter_context(
                tc.tile_pool(name='psum', bufs=6, space='PSUM'))
            pssm = ctx.enter_context(
                tc.tile_pool(name='psum_s', bufs=1, space='PSUM'))

            constA = cpool.tile([128, 183], F32)
            constB = cpool.tile([16, 2305], F32)

            # ---------- input loads: 4 quarter-DMAs per tile ----------
            QL = NPIX // 4
            xts = {}

            def issue_load(s, h):
                xt = xpool.tile([128, XT_LEN], F32R, tag='xt', name=f'x{s}{h}')
                nc.gpsimd.memset(xt[:, 0:DATA_OFF].bitcast(F32), 0)
                nc.gpsimd.memset(
                    xt[:, DATA_OFF + NPIX:XT_LEN].bitcast(F32), 0)
                xts[(s, h)] = xt
                return xt

            def issue_quarter(s, h, q):
                xt = xts[(s, h)]
                src = x_d[s, 128 * h:128 * (h + 1)].rearrange(
                    'c a b -> c (a b)')
                lo = DATA_OFF + QL * q
                nc.sync.dma_start(xt[:, lo:lo + QL],
                                  src[:, QL * q:QL * (q + 1)])

            # ---------- stats ----------
            # per tile: DVE bn_stats on quarters 0 (4.5 512-groups),
            # Scalar ACT accum on quarters 1-3. Combined into
            # (sum, sumsq) -> (mean, popvar) at finalize.
            stats = {}

            def stats_init(s, h):
                bn = bnpool.tile([128, 20, 6], F32, tag=f'bn{s}{h}',
                                 name=f'bn{s}{h}')
                sacc = bnpool.tile([128, 6], F32, tag=f'sa{s}{h}',
                                   name=f'sa{s}{h}')
                qacc = bnpool.tile([128, 6], F32, tag=f'qa{s}{h}',
                                   name=f'qa{s}{h}')
                st = {'bn': bn, 'sacc': sacc, 'qacc': qacc,
                      'nbn': 0, 'nsc': 0}
                stats[(s, h)] = st
                return st

            def stats_dve_quarter(s, h, q):
                """bn_stats over one quarter (2304 px) on DVE:
                4x512 + 1x256 groups."""
                xt = xts[(s, h)]
                xf = xt[:].bitcast(F32)
                st = stats[(s, h)]
                g0 = st['nbn']
                base = DATA_OFF + QL * q
                for g in range(4):
                    nc.vector.bn_stats(st['bn'][:, g0 + g, :],
                                       xf[:, base + 512 * g:
                                          base + 512 * (g + 1)])
                nc.vector.bn_stats(st['bn'][:, g0 + 4, :],
                                   xf[:, base + 2048:base + 2304])
                st['nbn'] += 5

            def stats_dve_half(s, h, q, part):
                """bn_stats over half a quarter (1152 px): 2x512+128."""
                xt = xts[(s, h)]
                xf = xt[:].bitcast(F32)
                st = stats[(s, h)]
                g0 = st['nbn']
                base = DATA_OFF + QL * q + (QL // 2) * part
                for g in range(2):
                    nc.vector.bn_stats(st['bn'][:, g0 + g, :],
                                       xf[:, base + 512 * g:
                                          base + 512 * (g + 1)])
                nc.vector.bn_stats(st['bn'][:, g0 + 2, :],
                                   xf[:, base + 1024:base + 1152])
                st['nbn'] += 3

            def stats_scalar_piece(s, h, q, part):
                """Scalar accum over half a quarter, part in {0,1}:
                Copy->sacc, Square->qacc."""
                xt = xts[(s, h)]
                xf = xt[:].bitcast(F32)
                st = stats[(s, h)]
                EL = QL // 2
                lo = DATA_OFF + QL * q + EL * part
                i = st['nsc']
                st['nsc'] += 1
                dump = dumppool.tile([128, EL], F32, tag='dump')
                nc.scalar.activation(dump[:], xf[:, lo:lo + EL], AF.Copy,
                                     accum_out=st['sacc'][:, i:i + 1])
                dump2 = dumppool.tile([128, EL], F32, tag='dump')
                nc.scalar.activation(dump2[:], xf[:, lo:lo + EL], AF.Square,
                                     accum_out=st['qacc'][:, i:i + 1])

            def stats_finalize(s, h):
                """Combine DVE bn + Scalar accums ->
                mv [128,2] = (mean, population var)."""
                st = stats[(s, h)]
                NBN = NPIX - st['nsc'] * (QL // 2)
                mvq = bnpool.tile([128, 2], F32, tag=f'mvq{s}{h}')
                nc.vector.bn_aggr(mvq[:], st['bn'][:, 0:st['nbn'], :])
                if st['nsc'] == 0:
                    # bn covered the whole tile: mvq is (mean, popvar)
                    return mvq
                # sums from scalar parts
                ssum = bnpool.tile([128, 2], F32, tag=f'ss{s}{h}')
                nsc = st['nsc']
                nc.vector.tensor_reduce(ssum[:, 0:1], st['sacc'][:, 0:nsc],
                                        axis=mybir.AxisListType.X,
                                        op=OP.add)
                nc.vector.tensor_reduce(ssum[:, 1:2], st['qacc'][:, 0:nsc],
                                        axis=mybir.AxisListType.X,
                                        op=OP.add)
                # totals: sum = mean_bn*NBN + ssum0 ; sumsq = (var_bn +
                # mean_bn^2)*NBN + ssum1
                tot = bnpool.tile([128, 2], F32, tag=f'tt{s}{h}')
                nc.vector.scalar_tensor_tensor(
                    tot[:, 0:1], mvq[:, 0:1], float(NBN), ssum[:, 0:1],
                    OP.mult, OP.add)
                e2 = bnpool.tile([128, 1], F32, tag=f'e2{s}{h}')
                nc.vector.scalar_tensor_tensor(
                    e2[:], mvq[:, 0:1], mvq[:, 0:1], mvq[:, 1:2],
                    OP.mult, OP.add)
                nc.vector.scalar_tensor_tensor(
                    tot[:, 1:2], e2[:], float(NBN), ssum[:, 1:2],
                    OP.mult, OP.add)
                # mv = (mean, popvar)
                mv = bnpool.tile([128, 2], F32, tag=f'mv{s}{h}')
                nc.vector.tensor_scalar_mul(mv[:], tot[:], 1.0 / NPIX)
                nc.vector.scalar_tensor_tensor(
                    mv[:, 1:2], mv[:, 0:1], mv[:, 0:1], mv[:, 1:2],
                    OP.mult, OP.subtract)
                nc.vector.tensor_scalar_mul(mv[:, 1:2], mv[:, 1:2], -1.0)
                return mv

            # ---------- dynamic weights ----------
            def sample_weights_a(s, mvs):
                """std/z/sigmoid/gap — vector+scalar small ops only."""
                g_cols = {}
                gap = smpool.tile([128, 2], F32, tag=f'gap{s}')
                for h in (0, 1):
                    m = mvs[h]
                    mean = m[:, 0:1]
                    std = smpool.tile([128, 1], F32, tag=f'std{s}{h}')
                    nc.scalar.activation(std[:], m[:, 1:2], AF.Sqrt,
                                         bias=constA[:, 182:183],
                                         scale=float(NPIX) / (NPIX - 1))
                    zt = smpool.tile([128, 1], F32, tag=f'zt{s}{h}')
                    nc.vector.tensor_tensor(zt[:], std[:],
                                            constA[:, 162 + h:163 + h],
                                            OP.mult)
                    nc.vector.scalar_tensor_tensor(
                        zt[:], mean, constA[:, 160 + h:161 + h], zt[:],
                        OP.mult, OP.add)
                    g = smpool.tile([128, 1], F32, tag=f'g{s}{h}')
                    nc.scalar.activation(g[:], zt[:], AF.Sigmoid)
                    g_cols[h] = g
                    nc.vector.tensor_tensor(gap[:, h:h + 1], g[:], mean,
                                            OP.mult)
                return g_cols, gap

            _warm = {}

            def pe_keepwarm(n):
                """Junk matmuls over landed x00 data: keep the PE
                busy-streak alive across weight-chain gaps so conv
                starts at full pstate."""
                if 'w' not in _warm:
                    jw = cpool.tile([128, 128], F32R)
                    nc.vector.tensor_scalar_mul(
                        jw[:], constA[:, 0:128], constA[:, 182:183])
                    _warm['w'] = jw
                for i in range(n):
                    ps = pspool.tile([128, CHUNK], F32, tag='cps')
                    nc.tensor.matmul(
                        ps[:], lhsT=_warm['w'][:],
                        rhs=xts[(0, 0)][:, DATA_OFF + 32 * i:
                                        DATA_OFF + 32 * i + CHUNK],
                        start=True, stop=True)

            def sample_weights_b(s, g_cols, gap):
                """hid/dyn PE matmuls + weff/negw."""
                p = pssm.tile([16, 2], F32, tag='hid')
                for h in (0, 1):
                    nc.tensor.matmul(p[:, h:h + 1],
                                     lhsT=constA[:, 128 + 16 * h:
                                                 144 + 16 * h],
                                     rhs=gap[:, h:h + 1],
                                     start=True, stop=True)
                hsum = smpool.tile([16, 1], F32, tag=f'hsum{s}')
                nc.vector.tensor_reduce(hsum[:], p[:],
                                        axis=mybir.AxisListType.X,
                                        op=OP.add)
                hid = smpool.tile([16, 1], F32, tag=f'hid{s}')
                nc.scalar.activation(hid[:], hsum[:], AF.Relu,
                                     bias=constB[:, 2304:2305], scale=1.0)
                per_half = {}
                for h in (0, 1):
                    pd = pssm.tile([128, 9], F32, tag='dyn', name=f'dyn{s}{h}')
                    for k in range(9):
                        nc.tensor.matmul(
                            pd[:, k:k + 1],
                            lhsT=constB[:, k * 256 + 128 * h:
                                        k * 256 + 128 * h + 128],
                            rhs=hid[:], start=True, stop=True)
                    b2g = smpool.tile([128, 9], F32, tag=f'b2g{s}{h}')
                    nc.vector.tensor_scalar_mul(
                        b2g[:], constA[:, 164 + 9 * h:173 + 9 * h],
                        g_cols[h][:])
                    weff = smpool.tile([128, 9], F32R, tag=f'weff{s}{h}')
                    nc.vector.scalar_tensor_tensor(
                        weff[:], pd[:], g_cols[h][:], b2g[:], OP.mult,
                        OP.add)
                    weff_f = weff[:].bitcast(F32)
                    negw = smpool.tile([128, 9], F32, tag=f'negw{s}{h}')
                    nc.vector.tensor_scalar_mul(negw[:], weff_f, -1.0)
                    per_half[h] = (weff, negw)
                return per_half

            def dg_build(s, h, weff, k, eng='s'):
                """One diag column-block build (Scalar, ~0.33us)."""
                key = (s, h)
                if key not in _dg_tiles:
                    _dg_tiles[key] = dgpool.tile([128, 9 * 128], F32R,
                                                 tag='dg',
                                                 name=f'dg{s}{h}')
                dg = _dg_tiles[key]
                weff_f = weff[:].bitcast(F32)
                if eng == 'v':
                    nc.vector.tensor_scalar_mul(
                        dg[:, 128 * k:128 * (k + 1)],
                        constA[:, 0:128], weff_f[:, k:k + 1])
                else:
                    nc.scalar.activation(
                        dg[:, 128 * k:128 * (k + 1)],
                        constA[:, 0:128], AF.Copy,
                        scale=weff_f[:, k:k + 1])
                return dg

            _dg_tiles = {}

            # ---------- conv ----------
            PE_COLS = RPE * W                      # flat cols of PE region
            n_chunks = (PE_COLS + CHUNK - 1) // CHUNK

            def conv_tile(s, h, weff, negw, fillers, hook=None,
                          last=False, rpe=RPE):
                """Emit PE chunks + Scalar drains (+fillers) + DVE
                region + fixups + out DMAs for one tile. hook() is
                emitted after the first 24-row group (PE-side work for
                the next sample's weight chain)."""
                xt = xts[(s, h)]
                xf = xt[:].bitcast(F32)
                xfr = xf.rearrange('p (r c) -> p r c', c=W)
                dg = _dg_tiles[(s, h)]
                weff_f = weff[:].bitcast(F32)
                out_flat = out_d[s, 128 * h:128 * (h + 1)].rearrange(
                    'c a b -> c (a b)')


                rdve = H - rpe

                def dve_region():
                    sv = svpool.tile([128, rdve * W], F32, tag='stg_dve',
                                     name=f'sv{s}{h}')
                    svr = sv[:].rearrange('p (r c) -> p r c', c=W)
                    a, b = rpe, H
                    nc.scalar.activation(
                        sv[:], xf[:, DATA_OFF + a * W:DATA_OFF + b * W],
                        AF.Copy, scale=weff_f[:, 4:5])
                    for k in (0, 1, 2, 3, 5, 6, 7, 8):
                        dy, dx = _tap(k)
                        if dx == 1:
                            co0, co1 = 0, W - 1
                        elif dx == -1:
                            co0, co1 = 1, W
                        else:
                            co0, co1 = 0, W
                        nc.vector.scalar_tensor_tensor(
                            svr[:, 0:rdve, co0:co1],
                            xfr[:, a + dy + GUARD_TOP:b + dy + GUARD_TOP,
                                co0 + dx:co1 + dx],
                            weff_f[:, k:k + 1],
                            svr[:, 0:rdve, co0:co1],
                            OP.mult, OP.add)
                    nc.gpsimd.dma_start(out_flat[:, a * W:b * W], sv[:])

                dve_region()
                # --- PE region: per-24-row stage groups, 6 chunks each
                for ga in range(0, rpe, DMA_ROWS):
                    gb = min(ga + DMA_ROWS, rpe)
                    gcols = (gb - ga) * W
                    stg = stpool.tile([128, gcols], F32, tag='stg_pe')
                    stgr = stg[:].rearrange('p (r c) -> p r c', c=W)
                    for c in range(gcols // CHUNK):
                        lo = ga * W + CHUNK * c
                        ps = pspool.tile([128, CHUNK], F32, tag='cps')
                        for k in range(9):
                            dy, dx = _tap(k)
                            off = DATA_OFF + lo + dy * W + dx
                            nc.tensor.matmul(
                                ps[:],
                                lhsT=dg[:, 128 * k:128 * (k + 1)],
                                rhs=xt[:, off:off + CHUNK],
                                start=(k == 0), stop=(k == 8))
                        nc.scalar.copy(
                            stg[:, CHUNK * c:CHUNK * (c + 1)], ps[:])
                        if fillers:
                            fillers.pop(0)()
                    for dy in (-1, 0, 1):
                        kp = (dy + 1) * 3 + 2
                        nc.vector.scalar_tensor_tensor(
                            stgr[:, 0:gb - ga, W - 1:W],
                            xfr[:, ga + dy + GUARD_TOP + 1:
                                gb + dy + GUARD_TOP + 1, 0:1],
                            negw[:, kp:kp + 1],
                            stgr[:, 0:gb - ga, W - 1:W],
                            OP.mult, OP.add)
                        km = (dy + 1) * 3
                        nc.vector.scalar_tensor_tensor(
                            stgr[:, 0:gb - ga, 0:1],
                            xfr[:, ga + dy + GUARD_TOP - 1:
                                gb + dy + GUARD_TOP - 1, W - 1:W],
                            negw[:, km:km + 1],
                            stgr[:, 0:gb - ga, 0:1],
                            OP.mult, OP.add)
                    if last and gb == rpe:
                        mid = (gb - ga) // 2 * W
                        nc.sync.dma_start(
                            out_flat[:, ga * W:ga * W + mid],
                            stg[:, 0:mid])
                        nc.sync.dma_start(
                            out_flat[:, ga * W + mid:gb * W],
                            stg[:, mid:(gb - ga) * W])
                    else:
                        nc.sync.dma_start(out_flat[:, ga * W:gb * W],
                                          stg[:])
                    if ga == 0 and hook is not None:
                        hook()

            # ================= emission =================
            for s, h in ((0, 0), (0, 1), (1, 0), (1, 1)):
                stats_init(s, h)
            for s, h in ((0, 0), (0, 1), (1, 0), (1, 1)):
                issue_load(s, h)
            # interleave s0 quarters so both halves land together
            for q in range(4):
                issue_quarter(0, 0, q)
                issue_quarter(0, 1, q)
            nc.sync.dma_start(constA[:], ca_d[:])
            nc.sync.dma_start(constB[:], cb_d[:])
            for q in range(4):
                issue_quarter(1, 0, q)
            for q in range(4):
                issue_quarter(1, 1, q)

            # s0 stats: DVE q0,q2 (+x00 q3); Scalar q1 (+x01 q3) —
            # both engines chew quarters concurrently as DMAs land
            stats_dve_quarter(0, 0, 0)
            stats_dve_quarter(0, 1, 0)
            for h in (0, 1):
                for part in (0, 1):
                    stats_scalar_piece(0, h, 1, part)
            stats_dve_quarter(0, 0, 2)
            stats_dve_quarter(0, 1, 2)
            stats_dve_half(0, 0, 3, 0)
            stats_scalar_piece(0, 0, 3, 1)
            stats_dve_half(0, 1, 3, 0)
            stats_scalar_piece(0, 1, 3, 1)
            mv00 = stats_finalize(0, 0)
            mv01 = stats_finalize(0, 1)
            g0, gap0 = sample_weights_a(0, {0: mv00, 1: mv01})
            ph0 = sample_weights_b(0, g0, gap0)
            # dg for half-0 first (head critical path), half-1 after;
            # split across Scalar/DVE (both idle here) to halve latency
            for k in range(9):
                dg_build(0, 0, ph0[0][0], k, eng='v' if k % 2 else 's')

            # fillers for conv(0,0): dg(0,1) + s1 scalar stats pieces
            def mk_dg_fillers(s, h, weff):
                return [lambda k=k: dg_build(s, h, weff, k)
                        for k in range(9)]

            def mk_stats_fillers(s, h):
                out = []
                for q in (1, 2, 3):
                    for part in (0, 1):
                        out.append(
                            lambda h=h, q=q, p=part:
                            stats_scalar_piece(s, h, q, p))
                return out

            f00 = mk_dg_fillers(0, 1, ph0[1][0]) \
                + mk_stats_fillers(1, 0)
            conv_tile(0, 0, ph0[0][0], ph0[0][1], f00)
            for f in f00:
                f()
            f00 = []
            # s1 stats: (1,0) Scalar q1-q3 (fillers above) + DVE q0;
            # (1,1) entirely DVE bn (fills its idle bubble). finalize +
            # weights part A before conv(0,1)'s DVE taps; PE part B as
            # a hook between conv(0,1) groups; dg builds as fillers.
            stats_dve_quarter(1, 0, 0)
            for q in range(4):
                stats_dve_quarter(1, 1, q)
            mv10 = stats_finalize(1, 0)
            mv11 = stats_finalize(1, 1)
            g1, gap1 = sample_weights_a(1, {0: mv10, 1: mv11})
            ph1 = {}

            def hook_b():
                ph1.update(sample_weights_b(1, g1, gap1))

            f01 = [(lambda: None) for _ in range(6)] \
                + [lambda k=k: dg_build(1, 0, ph1[0][0], k)
                   for k in range(9)] \
                + [lambda k=k: dg_build(1, 1, ph1[1][0], k)
                   for k in range(9)]
            conv_tile(0, 1, ph0[1][0], ph0[1][1], f01, hook=hook_b)
            for f in f01:
                f()
            conv_tile(1, 0, ph1[0][0], ph1[0][1], [])
            conv_tile(1, 1, ph1[1][0], ph1[1][1], [], last=True,
                      rpe=84)

    nc.compile()
    return nc


def _host_constants(cfc, w1, b1, w2, b2):
    A = np.zeros((128, 183), np.float32)
    A[:, 0:128] = np.eye(128, dtype=np.float32)
    w1T = np.ascontiguousarray(w1.T)              # [256, 16]
    A[:, 128:144] = w1T[:128]
    A[:, 144:160] = w1T[128:]
    A[:, 160] = cfc[0:128, 0]
    A[:, 161] = cfc[128:256, 0]
    A[:, 162] = cfc[0:128, 1]
    A[:, 163] = cfc[128:256, 1]
    b2r = b2.reshape(256, 9)
    A[:, 164:173] = b2r[0:128]
    A[:, 173:182] = b2r[128:256]
    A[:, 182] = EPS
    w2p = w2.reshape(256, 9, 16).transpose(1, 0, 2).reshape(2304, 16)
    B = np.zeros((16, 2305), np.float32)
    B[:, 0:2304] = w2p.T
    B[:, 2304] = b1
    return A, B


def kernel(x, cfc, w1, b1, w2, b2):
    global LAST_EXEC_NS, LAST_RESULTS
    _install_trace_hook_shim()
    from concourse.bass_utils import run_bass_kernel_spmd

    x = np.ascontiguousarray(x, dtype=np.float32)
    A, B = _host_constants(np.asarray(cfc, np.float32),
                           np.asarray(w1, np.float32),
                           np.asarray(b1, np.float32),
                           np.asarray(w2, np.float32),
                           np.asarray(b2, np.float32))

    if 'nc' not in _PROGRAM_CACHE:
        _PROGRAM_CACHE['nc'] = _build_program()
    nc = _PROGRAM_CACHE['nc']

    in_maps = [{'x': x[S_PER_CORE * i:S_PER_CORE * (i + 1)],
                'constA': A, 'constB': B} for i in range(CORES)]
    res = run_bass_kernel_spmd(nc, in_maps, list(range(CORES)))
    LAST_EXEC_NS = res.exec_time_ns
    LAST_RESULTS = res
    out = np.concatenate([res.results[i]['out'] for i in range(CORES)],
                         axis=0)
    return out.astype(np.float32, copy=False)
